# revision 1
# baseline (speedup 1.0000x reference)
"""Trainium2 kernel for nn_Deep_Tensor_Net (embedding_lookup).

Reference computation:
    W = einsum('aib,bjc,cka->ijk', A, Bf, C)     # [384, 384, 384]
    out[b, i] = W[i, x[b,0], x[b,1]]             # [524288, 384]

Factorization used here:
    out[b, :] = m_b @ A2t,  where
      m_b[f]    = Tg[jk_b, f],  f = b'*8 + a,  jk_b = x[b,0]*384 + x[b,1]
      Tg[jk, f] = sum_c Bf[b',j,c] * C[c,k,a]   # [147456, 64] table
      A2t[f, i] = A[a, i, b']                   # [64, 384]

Distribution: data-parallel over b across 8 cores (65536 rows each). Each
core's work is split into two halves of 32768 lookups; within a half,
np.unique guarantees <= 32768 distinct jk values, so gathers use int16
indices into a per-half sub-table Tsub = Tg[unique_jk] (<= 32768 x 64 f32).
The device gathers m rows in b-order with dma_gather, transposes pairs of
128-row tiles on the tensor engine, and matmuls mT against A2t (stationary
mT, moving A2t), accumulating out tiles [128 b, 384 i] in PSUM, which are
copied to SBUF staging and streamed to DRAM in b-order.

Slot mapping: gather slot i (per half) <-> b_half = (i % 128) * 256 + i // 128,
so each SBUF partition p owns output rows p*256 .. p*256+255 (contiguous
per-partition DMA writes).
"""
import os
import sys
import types

import numpy as np

# NTFF profile hook shim (the image's antenv lacks axon_hooks; the boot module
# has the ctypes implementation).
if "antenv.axon_hooks" not in sys.modules:
    try:
        from trn_agent_boot.trn_boot import _ntff_profile_via_ctypes

        _m = types.ModuleType("antenv.axon_hooks")
        _hook = _ntff_profile_via_ctypes("/opt/axon/libaxon_pjrt.so")
        _m.get_axon_ntff_profile_hook = lambda: _hook
        _m.set_axon_ntff_profile_hook = lambda h: None
        sys.modules["antenv.axon_hooks"] = _m
    except Exception:
        pass

import concourse.bass as bass
import concourse.mybir as mybir
import concourse.tile as tile
from concourse import bacc
from concourse.bass_utils import run_bass_kernel_spmd
from concourse.masks import make_identity

N = 384
R = 8
B = 524288
NCORES = 8
PER_CORE = B // NCORES          # 65536
HALF = PER_CORE // 2            # 32768
F = 64                          # m row length (f32 elems)
CHUNK = 512                     # gather slots per dma_gather instruction
NCHUNK = HALF // CHUNK          # 64
TPC = CHUNK // 128              # tiles per chunk = 4
STAGE_CHUNKS = 4                # chunks per staging buffer / output DMA
ROWS_PER_PART = HALF // 128     # 256 output rows owned by each partition/half

f32 = mybir.dt.float32
f32r = mybir.dt.float32r
i16 = mybir.dt.int16

MODE = os.environ.get("KERNEL_MM_MODE", "f32")   # "f32" (exact) or "f32r" (fast)


def _build_program():
    nc = bacc.Bacc("TRN2", target_bir_lowering=False, debug=True)
    tsub = nc.dram_tensor("tsub", [2, HALF, F], f32, kind="ExternalInput")
    idx_in = nc.dram_tensor("idx", [2, 128, HALF // 16], i16, kind="ExternalInput")
    a2_in = nc.dram_tensor("a2", [128, N], f32, kind="ExternalInput")
    y_out = nc.dram_tensor("y", [PER_CORE, N], f32, kind="ExternalOutput")
    # row b of y: half = b // HALF, partition p = (b % HALF) // 256, col = b % 256

    mm_dt = f32 if MODE == "f32" else f32r

    with tile.TileContext(nc) as tc:
        with tc.tile_pool(name="cpool", bufs=1) as cpool, \
             tc.tile_pool(name="idxp", bufs=2) as idxp, \
             tc.tile_pool(name="gp", bufs=4) as gp, \
             tc.tile_pool(name="mtp", bufs=4) as mtp, \
             tc.tile_pool(name="stp", bufs=2) as stp, \
             tc.tile_pool(name="ptp", bufs=2, space="PSUM") as ptp, \
             tc.tile_pool(name="pop", bufs=4, space="PSUM") as pop:
            ident = cpool.tile([128, 128], f32)
            make_identity(nc, ident[:])
            a2_raw = cpool.tile([128, N], f32)
            nc.sync.dma_start(out=a2_raw[:], in_=a2_in[:])
            if MODE == "f32":
                a2t = a2_raw
            else:
                a2t = cpool.tile([128, N], f32r)
                nc.vector.tensor_copy(out=a2t[:], in_=a2_raw[:])

            for half in range(2):
                idx_t = idxp.tile([128, HALF // 16], i16, tag="idx")
                nc.sync.dma_start(out=idx_t[:], in_=idx_in[half])
                for sc in range(NCHUNK // STAGE_CHUNKS):      # 16 stage groups
                    stage = stp.tile([128, STAGE_CHUNKS * TPC, N], f32, tag="st")
                    for cc in range(STAGE_CHUNKS):
                        c = sc * STAGE_CHUNKS + cc
                        g_t = gp.tile([128, TPC, F], f32, tag="g")
                        nc.gpsimd.dma_gather(
                            out_ap=g_t[:],
                            in_ap=tsub[half],
                            idxs_ap=idx_t[:, c * (CHUNK // 16):(c + 1) * (CHUNK // 16)],
                            num_idxs=CHUNK,
                            num_idxs_reg=CHUNK,
                            elem_size=F,
                        )
                        for pair in range(TPC // 2):
                            tp = ptp.tile([128, 128], f32, tag="tp")
                            nc.tensor.transpose(
                                out=tp[:],
                                in_=g_t[:, 2 * pair:2 * pair + 2, :].opt(),
                                identity=ident[:],
                            )
                            mt = mtp.tile([128, 128], mm_dt, tag="mt")
                            nc.vector.tensor_copy(out=mt[:], in_=tp[:])
                            for hh in range(2):
                                t = 2 * pair + hh
                                po = pop.tile([128, N], f32, tag="po")
                                nc.tensor.matmul(
                                    out=po[:],
                                    lhsT=mt[64 * hh:64 * hh + 64, :],
                                    rhs=a2t[64 * hh:64 * hh + 64, :],
                                    start=True, stop=True,
                                )
                                nc.vector.tensor_copy(
                                    out=stage[:, cc * TPC + t, :], in_=po[:]
                                )
                    # stage holds rows b_half = p*256 + sc*16 + s for s in 0..16
                    # -> per partition: 16 consecutive rows at p*256 + sc*16
                    yv = y_out[half * HALF:(half + 1) * HALF, :]      # [32768, 384]
                    # target AP: [128 p, 16 rows, 384] with row = p*256 + sc*16 + s
                    dst = yv.rearrange("(p r) n -> p r n", p=128)     # [128, 256, 384]
                    nc.sync.dma_start(
                        out=dst[:, sc * STAGE_CHUNKS * TPC:(sc + 1) * STAGE_CHUNKS * TPC, :],
                        in_=stage[:],
                    )
    nc.finalize()
    return nc


def _wrap_idx(inv):
    """int16 stream [HALF] -> wrapped SBUF layout [128, HALF//16]:
    index i lives at [i % 16, i // 16], replicated to 128 partitions."""
    w = inv.reshape(HALF // 16, 16).T.astype(np.int16)   # [16, HALF//16]
    return np.tile(w, (8, 1))


def kernel(A, Bf, C, x):
    A = np.asarray(A, dtype=np.float32)
    Bf = np.asarray(Bf, dtype=np.float32)
    C = np.asarray(C, dtype=np.float32)
    x = np.asarray(x, dtype=np.int32)

    # Tables (f64 intermediate for accuracy, stored f32)
    T4 = np.einsum(
        "bjc,cka->jkba", Bf.astype(np.float64), C.astype(np.float64)
    ).astype(np.float32)                                   # [384, 384, 8, 8]
    Tg = np.ascontiguousarray(T4.reshape(N * N, F))        # [147456, 64]
    A2t = np.ascontiguousarray(
        A.astype(np.float64).transpose(2, 0, 1).reshape(F, N)
    ).astype(np.float32)                                   # [64, 384]
    a2_stack = np.concatenate([A2t, A2t], axis=0)          # [128, 384]

    jk = x[:, 0].astype(np.int64) * N + x[:, 1].astype(np.int64)   # [B]

    # slot mapping within a half: slot i <-> b_half = (i % 128) * 256 + i // 128
    # equivalently: idx_stream[i] = inv[bmap[i]]
    i_arr = np.arange(HALF)
    bmap = (i_arr % 128) * ROWS_PER_PART + i_arr // 128

    in_maps = []
    for d in range(NCORES):
        tsub = np.zeros((2, HALF, F), dtype=np.float32)
        idxw = np.empty((2, 128, HALF // 16), dtype=np.int16)
        for h in range(2):
            seg = jk[d * PER_CORE + h * HALF: d * PER_CORE + (h + 1) * HALF]
            uniq, inv = np.unique(seg, return_inverse=True)
            tsub[h, :len(uniq)] = Tg[uniq]
            idxw[h] = _wrap_idx(inv[bmap].astype(np.int16))
        in_maps.append({"tsub": tsub, "idx": idxw, "a2": a2_stack})

    nc = _build_program()
    trace = os.environ.get("KERNEL_TRACE", "0") == "1"
    res = run_bass_kernel_spmd(
        nc, in_maps, core_ids=list(range(NCORES)), trace=trace,
        tmpdir=os.environ.get("KERNEL_TRACE_DIR") or None,
    )
    if trace:
        kernel.last_exec_time_ns = res.exec_time_ns
        kernel.last_results = res

    out = np.concatenate([res.results[d]["y"] for d in range(NCORES)], axis=0)
    return out
